# revision 77
# baseline (speedup 1.0000x reference)
"""Trainium2 Bass kernel for nn_MixedLipMlp (soft-MoE MLP with Lipschitz gate).

Strategy: data-parallel over batch B=4096 across 8 NeuronCores (512 rows each,
expert weights + gate replicated). Activations live feature-major (features on
partitions, batch on the free dim) for layers 0/1; layer 2 stays feature-major
too (out [96, 512] = experts o-major) and the coefficient mix is ONE one-hot
partition-sum matmul, so the output DMA is 12 large packets instead of 128
tiny ones (the v1 batch-major mix cost ~4.3us of DMA tail).

v3 changes vs the 92.7us v1 baseline:
  - layer 2 feature-major + one-hot mix matmul (kills the coeffB machinery,
    the per-bt prod/reduce, and the slow [128, 48] output DMA).
  - constant-bias fast path: the reference initializes expert biases to a
    constant (b = 0.01); when all-equal, coeff @ b == b (softmax sums to 1),
    so the bias folds into the ELU epilogue as a per-partition ACT bias and
    the 8 bias matmuls + bpack DMA disappear. General-bias fallback keeps the
    v1 bias-matmul path.
  - k-major (slab-major) h-passes in layers 0/1: the DVE produces the
    coeff-scaled rhs tiles just-in-time, one k-slab ahead of the PE, instead
    of needing all slabs of an expert up front.
  - mid-layer ELU = exp on ACT + relu on ACT + min on DVE; the final k-group
    of each layer runs m-outer so PSUM banks close staggered and the ELU
    pipeline overlaps the remaining matmuls.
  - DMA: critical inputs (gate weights, z, c) first on sync/scalar; the big
    weight streams are held behind a tiny cross-queue dependency on z so they
    don't contend with the gate-critical transfers, then stream need-ordered
    (w0 before w1 halves, wz1/w2 between).
"""

import os
import sys

if "/opt/trn_rl_repo" not in sys.path:
    sys.path.insert(0, "/opt/trn_rl_repo")

# recover cleanly if a previous process left the NeuronCores wedged
os.environ.setdefault("NEURON_RT_RESET_CORES", "1")

import numpy as np
import ml_dtypes

F8E3NP = ml_dtypes.float8_e3m4
WSCALE = 64.0  # e3m4 weight scaling (undone by the ELU epilogue)

# Problem dimensions (hardcoded; must match the grader's setup_inputs()).
B = 4096
NCORES = 8
BS = B // NCORES  # 512 batch rows per core = matmul free dim
LATENT = 64
INPUT_SIZE = 256
IN_DIM = LATENT + INPUT_SIZE  # 320
HIDDEN = 512
ACTIONS = 12
E = 8
GATE_H = 128
INTER = HIDDEN + LATENT  # 576

NK0 = 2   # layer0: c has 256 rows = 2 k-slabs
NK12 = 4  # layers1,2: h has 512 rows = 4 k-slabs
N_M = HIDDEN // 128  # 4 output m-tiles for layers 0/1
NA = E * ACTIONS  # 96: layer-2 outputs packed o-major (col = o*8+e)

TRACE = False
LAST_EXEC_NS = None
LAST_RESULTS = None


def _build_nc(bias_const):
    import concourse.mybir as mybir
    from concourse import bacc
    from concourse.tile import TileContext

    dt = mybir.dt
    F32 = dt.float32
    F16 = dt.float16
    F8E3 = dt.float8e3
    AF = mybir.ActivationFunctionType
    OP = mybir.AluOpType
    # l0/l1 weights ship as e3m4 (4-bit mantissa) scaled by 64: halves the
    # weight DMA at unchanged PE speed (matmul speed keys on the moving
    # operand's dtype; fp8 lhsT x fp16 rhs is legal), costs ~1e-2 of the
    # 2e-2 error budget. The ELU epilogue unscales with ACT scale=1/64.
    WSCALE_INV = 1.0 / 64.0

    nc = bacc.Bacc("TRN2", target_bir_lowering=False)

    # ---- DRAM I/O ------------------------------------------------------
    # ALL critical inputs ride in ONE consolidated tensor on the gpsimd
    # (SWDGE) queue: sync/scalar DMAs share a single hardware DMA engine
    # (~26GB/s serialized), while SWDGE fans packets across 8 engines.
    # cols 0:542 gatepack: gw0a(0:128,rows<64) gw0b(128:256) gw0c(256:384)
    #   gw1(384:512) gw2(512:520), Lipschitz-folded on the host; 520:530
    #   the five f32 biases (gb0 gb1 gb2 b0c b1c) as f16 bit pairs
    #   (bitcast on device); 530:542 rows<96: sel12.
    # cols 542:1054 rows<64: xinB (zT); 1054:1566 c slab0; 1566:2078 c
    # slab1 (ordered so the first DMA chunk carries everything the gate
    # chain's first matmuls need).
    d_inp = nc.dram_tensor("inpack", [128, 2078], F16, kind="ExternalInput")
    # selpack cols: sel8(0:1024) sel96(1024:1120)
    d_selp = nc.dram_tensor("selpack", [E, 1120], F16, kind="ExternalInput")
    d_wz0 = nc.dram_tensor("wz0", [128, E // 2 * HIDDEN], F8E3,
                           kind="ExternalInput")
    d_wz1 = nc.dram_tensor("wz1", [128, E // 2 * HIDDEN], F8E3,
                           kind="ExternalInput")
    # w2pack cols: w2h slabs (0:384) all 128 rows; w2z+b2 (384:480) rows<65
    d_w2 = nc.dram_tensor("w2pack", [128, 480], F16, kind="ExternalInput")
    d_w0h = nc.dram_tensor("w0hcat", [128, E * NK0 * HIDDEN], F8E3,
                           kind="ExternalInput")
    d_w1h = nc.dram_tensor("w1hcat", [128, E * NK12 * HIDDEN], F8E3,
                           kind="ExternalInput")
    d_bp = nc.dram_tensor("bpack", [E, 2 * HIDDEN], F16, kind="ExternalInput")
    d_out = nc.dram_tensor("outF", [ACTIONS, BS], F32, kind="ExternalOutput")

    mm = nc.tensor.matmul
    SL0 = NK0 * HIDDEN    # 1024 cols per l0 expert slab block
    SL1 = NK12 * HIDDEN   # 2048 cols per l1 expert slab block

    with TileContext(nc) as tc:
        from contextlib import ExitStack

        with ExitStack() as ctx:
            pers = ctx.enter_context(tc.tile_pool(name="pers", bufs=1))
            sca = ctx.enter_context(tc.tile_pool(name="sca", bufs=10))
            etmp = ctx.enter_context(tc.tile_pool(name="etmp", bufs=4))

            # ---- DMA: everything on the gpsimd/SWDGE ring, need-ordered --
            inp = pers.tile([128, 2078], F16, tag="inp")
            nc.gpsimd.dma_start(out=inp[:, 0:1566], in_=d_inp[:, 0:1566])
            nc.gpsimd.dma_start(out=inp[:, 1566:2078],
                                in_=d_inp[:, 1566:2078])
            gate = inp[:, 0:542]
            xinB = inp[0:LATENT, 542:1054]
            xinA = inp[:, 1054:2078]

            # selector one-hots: small 8-packet transfer on the otherwise
            # idle scalar queue
            selp = pers.tile([E, 1120], F16, tag="selp")
            nc.scalar.dma_start(out=selp, in_=d_selp[:, :])
            sel8 = selp[:, 0:1024]
            sel96 = selp[:, 1024:1120]

            w0hcat = pers.tile([128, E * SL0], F8E3, tag="w0hcat")
            w1hcat = pers.tile([128, E * SL1], F8E3, tag="w1hcat")
            wz0 = pers.tile([128, E // 2 * HIDDEN], F8E3, tag="wz0")
            wz1 = pers.tile([128, E // 2 * HIDDEN], F8E3, tag="wz1")
            w2p = pers.tile([128, 480], F16, tag="w2p")

            # weight stream, held until the inputs land (the DMA hw round-
            # robins packets across outstanding transfers, so an early
            # weight stream starves the critical inputs). Tile hoists any
            # dma_start with no dependency, so EVERY gated dma gets its own
            # compute-op touch (RAW on the inputs, WAW into its own output
            # region). Ring order = need order: w0 chunks, z packs, w1
            # expert chunks, w2.
            in8 = inp[0:128, 0:2].bitcast(F8E3)

            def _gp_gated(out_ap, in_ap, touch_sl):
                nc.gpsimd.tensor_copy(out=touch_sl, in_=in8)
                nc.gpsimd.dma_start(out=out_ap, in_=in_ap)

            for c0 in range(4):
                sl = slice(2 * c0 * SL0, 2 * (c0 + 1) * SL0)
                _gp_gated(w0hcat[:, sl], d_w0h[:, sl],
                          w0hcat[0:128, sl.start:sl.start + 4])
            _gp_gated(wz0, d_wz0[:, :], wz0[0:128, 0:4])
            _gp_gated(wz1, d_wz1[:, :], wz1[0:128, 0:4])
            for c1 in range(4):
                sl = slice(2 * c1 * SL1, 2 * (c1 + 1) * SL1)
                _gp_gated(w1hcat[:, sl], d_w1h[:, sl],
                          w1hcat[0:128, sl.start:sl.start + 4])
            nc.gpsimd.tensor_copy(out=w2p[0:128, 0:2],
                                  in_=inp[0:128, 0:2])
            nc.gpsimd.dma_start(out=w2p, in_=d_w2[:, :])
            if not bias_const:
                bp = pers.tile([E, 2 * HIDDEN], F16, tag="bp")
                nc.gpsimd.tensor_copy(out=bp[0:E, 0:2], in_=inp[0:E, 0:2])
                nc.gpsimd.dma_start(out=bp, in_=d_bp[:, :])
                b0sb = bp[:, 0:HIDDEN]
                b1sb = bp[:, HIDDEN:]

            gw0t = [gate[0:64, 0:128], gate[:, 128:256], gate[:, 256:384]]
            gw1t = gate[:, 384:512]
            gw2t = gate[:, 512:520]
            gbp = gate[:, 520:530].bitcast(F32)  # [128, 5] f32 biases
            sel12 = gate[0:NA, 530:542]
            sel8 = selp[:, 0:1024]
            sel96 = selp[:, 1024:1120]
            gb0 = gbp[:, 0:1]
            gb1 = gbp[:, 1:2]
            gb2 = gbp[0:E, 2:3]
            b0c = gbp[:, 3:4]
            b1c = gbp[:, 4:5]
            # 64*b1 as a per-partition f32 scalar, for the DVE-relu variant
            # of the last l1 ELU (relu(64y') = 64*relu(y'), so the x64 PSUM
            # scale folds through the max)
            b1c64 = pers.tile([128, 1], F32, tag="b1c64")
            nc.scalar.activation(out=b1c64, in_=b1c, func=AF.Copy,
                                 scale=WSCALE)
            xc = [xinA[:, 0:512], xinA[:, 512:1024]]
            w0h = [w0hcat[:, e * SL0:(e + 1) * SL0] for e in range(E)]
            w1h = [w1hcat[:, e * SL1:(e + 1) * SL1] for e in range(E)]
            w2h = [w2p[:, k * NA:(k + 1) * NA] for k in range(NK12)]
            w2z = w2p[0:LATENT + 1, 384:480]

            # ---- constants + on-device z expansion -----------------------
            ones_blk = pers.tile([128, 128], F16, tag="ones_blk")
            nc.vector.memset(ones_blk, 1.0)
            warm_rhs = pers.tile([128, BS], F16, tag="warm_rhs")
            nc.vector.memset(warm_rhs, 0.0)
            # z duplicated into both row halves for the zsf scalings; ones
            # row appended for the l2 bias
            xz2 = pers.tile([128, BS], F16, tag="xz2")
            nc.vector.tensor_copy(out=xz2[0:LATENT, :], in_=xinB)
            nc.vector.tensor_copy(out=xz2[LATENT:128, :], in_=xinB)
            xzo = pers.tile([LATENT + 1, BS], F16, tag="xzo")
            nc.vector.tensor_copy(out=xzo[0:LATENT, :], in_=xinB)
            nc.vector.memset(xzo[LATENT:LATENT + 1, :], 1.0)

            # ---- gate + softmax + coefficient broadcasts -----------------
            # the whole chain is column-split into four 128-wide quarters:
            # its serial latency (mm -> elu -> mm -> ... -> coeff) is the
            # longest PE-idle stretch of the kernel, and quarter-stages
            # pipeline across the PE/ACT/DVE engines
            QW = BS // 4
            quarters = [slice(QW * i, QW * (i + 1)) for i in range(4)]
            with tc.tile_pool(name="ps_g", bufs=4, space="PSUM") as ps_g:

                # trip the PE activity monitor before the gate chain; 7
                # fillers keep the PE warm until the inputs land (bufs=2 so
                # they pipeline instead of serializing on the bank drain)
                for _ in range(7):
                    pw = ps_g.tile([128, BS], F32, tag="warm", bufs=2,
                                   name=f"warm{nc.next_id()}")
                    mm(pw, ones_blk, warm_rhs, start=True, stop=True)

                def gate_elu_q(ps, bias, out, sl):
                    # elu(y) = min(exp(y)-1, relu(y)); exp on ACT and relu
                    # on DVE run concurrently (latency-critical chain)
                    ex = etmp.tile([ps.shape[0], QW], F16, tag="elu_exp",
                                   name=f"gex{nc.next_id()}")
                    nc.scalar.activation(out=ex, in_=ps, func=AF.Exp,
                                         bias=bias)
                    rl = etmp.tile([ps.shape[0], QW], F16, tag="elu_relu",
                                   name=f"grl{nc.next_id()}")
                    nc.vector.tensor_scalar(rl, ps, bias, 0.0, OP.add, OP.max)
                    nc.vector.scalar_tensor_tensor(
                        out=out[:, sl], in0=ex, scalar=1.0, in1=rl,
                        op0=OP.subtract, op1=OP.min,
                    )

                h0g = pers.tile([GATE_H, BS], F16, tag="h0g")
                h1g = pers.tile([GATE_H, BS], F16, tag="h1g")
                expl = pers.tile([E, BS], F16, tag="expl")
                bcR = pers.tile([128, BS], F32, tag="bcR")
                coeffT = pers.tile([E, BS], F16, tag="coeffT")
                rhs0 = [xinB, xc[0], xc[1]]
                psg0, psg1, pslg, pssum = [], [], [], []
                for qi, sl in enumerate(quarters):
                    p = ps_g.tile([GATE_H, QW], F32, tag="g", bufs=3,
                                  name=f"psg0{qi}")
                    for k in range(3):
                        mm(p, gw0t[k], rhs0[k][:, sl],
                           start=(k == 0), stop=(k == 2))
                    psg0.append(p)
                for qi, sl in enumerate(quarters):
                    gate_elu_q(psg0[qi], gb0, h0g, sl)
                for qi, sl in enumerate(quarters):
                    p = ps_g.tile([GATE_H, QW], F32, tag="g", bufs=3,
                                  name=f"psg1{qi}")
                    mm(p, gw1t, h0g[:, sl], start=True, stop=True)
                    psg1.append(p)
                for qi, sl in enumerate(quarters):
                    gate_elu_q(psg1[qi], gb1, h1g, sl)
                for qi, sl in enumerate(quarters):
                    p = ps_g.tile([E, QW], F32, tag="lg", name=f"pslg{qi}",
                                  bufs=2)
                    mm(p, gw2t, h1g[:, sl], start=True, stop=True)
                    pslg.append(p)
                # softmax over the 8 expert partitions (logits bounded by the
                # lip constraint, no max subtraction needed)
                for qi, sl in enumerate(quarters):
                    nc.scalar.activation(out=expl[:, sl], in_=pslg[qi],
                                         func=AF.Exp, bias=gb2)
                    p = ps_g.tile([128, QW], F32, tag="sum", name=f"pss{qi}",
                                  bufs=1)
                    mm(p, ones_blk[:E, :], expl[:, sl], start=True, stop=True)
                    pssum.append(p)
                for qi, sl in enumerate(quarters):
                    nc.vector.reciprocal_approx_fast(out=bcR[:, sl],
                                                     in_=pssum[qi])
                    nc.vector.tensor_mul(coeffT[:, sl], expl[:, sl],
                                         bcR[:E, sl])

            # broadcast each normalized coeff row to all 128 partitions.
            # Only the first three are emitted up front; the rest
            # interleave with l0's first expert groups so the PE never
            # waits on the PSUM-pool drain (the ACT copies).
            acc0_ctx = tc.tile_pool(name="ps_acc0", bufs=4, space="PSUM")
            ps_acc0 = acc0_ctx.__enter__()
            bc_ctx = tc.tile_pool(name="ps_bc", bufs=3, space="PSUM")
            ps_bc = bc_ctx.__enter__()
            bcE = []

            def emit_bcE(e):
                pb = ps_bc.tile([128, BS], F32, tag="bc", name=f"pbc{e}",
                                bufs=3)
                mm(pb, sel8[:, 128 * e: 128 * (e + 1)], coeffT,
                   start=True, stop=True)
                t = pers.tile([128, BS], F16, tag=f"bcE{e}")
                nc.scalar.activation(out=t, in_=pb, func=AF.Copy)
                bcE.append(t)

            for e in range(3):
                emit_bcE(e)
            cs = [[None] * E for _ in range(NK0)]
            zsf = [None] * E

            def z_pass(wz, psl, start):
                # row-paired z matmuls: two experts concurrently in disjoint
                # PE row groups; T1/T2 packing swaps experts between groups
                # so each group covers all 4 m-slices (top -> banks {0,1},
                # bottom -> banks {2,3}).
                for p in range(E // 2):
                    for t_ in range(2):
                        base = p * HIDDEN + t_ * 256
                        etop = 2 * p + t_
                        ebot = 2 * p + 1 - t_
                        st = start and p == 0 and t_ == 0
                        for mi in range(2):
                            mm(psl[mi],
                               wz[:LATENT, base + 128 * mi: base + 128 * (mi + 1)],
                               zsf[etop][:LATENT, :],
                               start=st, stop=False)
                            mm(psl[2 + mi],
                               wz[LATENT:, base + 128 * mi: base + 128 * (mi + 1)],
                               zsf[ebot][LATENT:, :],
                               start=st, stop=False)

            def moe_elu(psl_m, bias, out_tag, min_eng=None):
                # elu(y/64 + b) = min(exp(.)-1, relu(.)); exp and relu both
                # on ACT (throughput path; the DVE is loaded with the
                # scalings), min on Pool by default (DVE for the l2-latency-
                # critical last tile). scale=1/64 undoes the e3m4 weight
                # scaling.
                ex = etmp.tile([128, BS], F16, tag="elu_exp",
                               name=f"mex{nc.next_id()}")
                nc.scalar.activation(out=ex, in_=psl_m, func=AF.Exp,
                                     bias=bias if bias is not None else 0.0,
                                     scale=WSCALE_INV)
                rl = etmp.tile([128, BS], F16, tag="elu_relu",
                               name=f"mrl{nc.next_id()}")
                nc.scalar.activation(out=rl, in_=psl_m, func=AF.Relu,
                                     bias=bias if bias is not None else 0.0,
                                     scale=WSCALE_INV)
                h = pers.tile([128, BS], F16, tag=out_tag, name=out_tag)
                (min_eng or nc.vector).scalar_tensor_tensor(
                    out=h, in0=ex, scalar=1.0, in1=rl,
                    op0=OP.subtract, op1=OP.min,
                )
                return h

            def moe_elu_fast(psl_m, bias64, bias, out_tag):
                # latency-optimized variant for the tile the l2 chain waits
                # on: relu on DVE (parallel with the ACT exp) in the scaled
                # domain — relu(64y'+64b) = 64*relu(y'+b)
                ex = etmp.tile([128, BS], F16, tag="elu_exp",
                               name=f"fex{nc.next_id()}")
                nc.scalar.activation(out=ex, in_=psl_m, func=AF.Exp,
                                     bias=bias if bias is not None else 0.0,
                                     scale=WSCALE_INV)
                rl64 = etmp.tile([128, BS], F16, tag="elu_relu",
                                 name=f"fr64{nc.next_id()}")
                if bias64 is not None:
                    nc.vector.tensor_scalar(rl64, psl_m, bias64, 0.0,
                                            OP.add, OP.max)
                else:
                    nc.vector.tensor_scalar(rl64, psl_m, 0.0, None, OP.max)
                rl = etmp.tile([128, BS], F16, tag="elu_relu",
                               name=f"frl{nc.next_id()}")
                nc.vector.tensor_scalar(rl, rl64, WSCALE_INV, None, OP.mult)
                h = pers.tile([128, BS], F16, tag=out_tag, name=out_tag)
                nc.vector.scalar_tensor_tensor(
                    out=h, in0=ex, scalar=1.0, in1=rl,
                    op0=OP.subtract, op1=OP.min,
                )
                return h

            def h_block(wh, hs_tiles, kis, psl, start):
                # e-major slab groups: each scaled rhs tile is consumed
                # right after the DVE makes it
                for idx, ki in enumerate(kis):
                    for e in range(E):
                        for m in range(N_M):
                            mm(psl[m], wh[e][:, ki * HIDDEN + 128 * m:
                                             ki * HIDDEN + 128 * (m + 1)],
                               hs_tiles[ki][e],
                               start=(start and idx == 0 and e == 0),
                               stop=False)

            def h_close(wh, hs_tiles, ki, psl, bsb, bias, htag,
                        last_min_eng=None):
                # final k-group m-outer: banks close staggered so the ELUs
                # pipeline while the remaining banks still accumulate
                hts = []
                for m in range(N_M):
                    for e in range(E):
                        last = e == E - 1
                        if last and bsb is not None:
                            mm(psl[m], wh[e][:, ki * HIDDEN + 128 * m:
                                             ki * HIDDEN + 128 * (m + 1)],
                               hs_tiles[ki][e], start=False, stop=False)
                            mm(psl[m], bsb[:, 128 * m: 128 * (m + 1)], coeffT,
                               start=False, stop=True)
                        else:
                            mm(psl[m], wh[e][:, ki * HIDDEN + 128 * m:
                                             ki * HIDDEN + 128 * (m + 1)],
                               hs_tiles[ki][e], start=False, stop=last)
                    if m == N_M - 1 and last_min_eng is not None:
                        hts.append(moe_elu_fast(
                            psl[m], b1c64 if bias is not None else None,
                            bias, f"{htag}{m}"))
                    else:
                        hts.append(moe_elu(psl[m], bias, f"{htag}{m}"))
                return hts

            def h_close_e(wh, hs_tiles, ki, psl, bsb, bias, htag):
                # e-major final k-group: each scaled tile is consumed as
                # soon as the DVE makes it (an m-outer close here would
                # demand every expert's tile at once and stall the PE at
                # the DVE's production rate); ELUs bunch at the end.
                for e in range(E):
                    last = e == E - 1 and bsb is None
                    for m in range(N_M):
                        mm(psl[m], wh[e][:, ki * HIDDEN + 128 * m:
                                         ki * HIDDEN + 128 * (m + 1)],
                           hs_tiles[ki][e], start=False, stop=last)
                if bsb is not None:
                    for m in range(N_M):
                        mm(psl[m], bsb[:, 128 * m: 128 * (m + 1)], coeffT,
                           start=False, stop=True)
                return [moe_elu(psl[m], bias, f"{htag}{m}")
                        for m in range(N_M)]

            # ---- MoE layer 0 -------------------------------------------
            # l0 runs c-slab0 FIRST: its expert groups are interleaved (in
            # PE program order) with the remaining bcE broadcasts, the z
            # pass in the middle, c-slab1 last. Scaled-input production
            # order mirrors the consumption order exactly.
            ps_l0 = [ps_acc0.tile([128, BS], F32, tag="acc", name=f"psl0_{m}")
                     for m in range(N_M)]
            for e in range(E):
                if e + 3 < E:
                    emit_bcE(e + 3)
                t = sca.tile([128, BS], F16, tag="s", name=f"c0_{e}")
                nc.vector.tensor_mul(t, xc[0], bcE[e])
                cs[0][e] = t
                for m in range(N_M):
                    mm(ps_l0[m], w0h[e][:, 128 * m: 128 * (m + 1)],
                       cs[0][e], start=(e == 0), stop=False)
            # batch-major coeff for the l2 mix: cX[o*8+e, b] = coeff[e, b]
            pcx = ps_bc.tile([128, BS], F32, tag="bc", name="pcx", bufs=3)
            mm(pcx[0:NA, :], sel96, coeffT, start=True, stop=True)
            cX = pers.tile([NA, BS], F16, tag="cX")
            nc.scalar.activation(out=cX, in_=pcx[0:NA, :], func=AF.Copy)
            bc_ctx.__exit__(None, None, None)

            # coeff-scaled z per expert (z duplicated in both 64-row halves
            # so an expert can ride either PE row group); shared by l0+l1
            for e in range(E):
                t = pers.tile([128, BS], F16, tag=f"zsf{e}")
                nc.vector.tensor_mul(t, xz2, bcE[e])
                zsf[e] = t
            z_pass(wz0, ps_l0, start=False)
            for e in range(E):
                t = sca.tile([128, BS], F16, tag="s", name=f"c1_{e}")
                nc.vector.tensor_mul(t, xc[1], bcE[e])
                cs[1][e] = t
            if bias_const:
                h0m = h_close_e(w0h, cs, 1, ps_l0, None, b0c, "h0m")
            else:
                h0m = h_close_e(w0h, cs, 1, ps_l0, b0sb, None, "h0m")

            # l1 scaled h inputs, k-major
            hs1 = [[None] * E for _ in range(NK12)]
            for ki in range(NK12):
                for e in range(E):
                    t = sca.tile([128, BS], F16, tag="s", name=f"h{ki}_{e}")
                    nc.vector.tensor_mul(t, h0m[ki], bcE[e])
                    hs1[ki][e] = t

            acc1_ctx = tc.tile_pool(name="ps_acc1", bufs=4, space="PSUM")
            ps_acc1 = acc1_ctx.__enter__()
            ps_l1 = [ps_acc1.tile([128, BS], F32, tag="acc", name=f"psl1_{m}")
                     for m in range(N_M)]
            z_pass(wz1, ps_l1, start=True)
            h_block(w1h, hs1, [0, 1, 2], ps_l1, start=False)
            if bias_const:
                h1m = h_close(w1h, hs1, 3, ps_l1, None, b1c, "h1m",
                              last_min_eng=nc.vector)
            else:
                h1m = h_close(w1h, hs1, 3, ps_l1, b1sb, None, "h1m",
                              last_min_eng=nc.vector)
            acc1_ctx.__exit__(None, None, None)
            acc0_ctx.__exit__(None, None, None)

            # ---- MoE layer 2, feature-major: y96[o*8+e, b], then the mix
            # is one elementwise multiply + one one-hot partition-sum
            # matmul per batch half (halves pipeline the serial
            # mul->mm->copy->dma tail)
            with tc.tile_pool(name="ps_l2", bufs=1, space="PSUM") as ps_l2:
                y96 = ps_l2.tile([NA, BS], F32, tag="y96", name="y96")
                mm(y96, w2z, xzo, start=True, stop=False)
                for k in range(NK12 - 1):
                    mm(y96, w2h[k], h1m[k], start=False, stop=False)
                ymix = pers.tile([NA, BS], F16, tag="ymix")
                out12 = ps_l2.tile([ACTIONS, BS], F32, tag="o12", name="o12")
                acto = pers.tile([ACTIONS, BS], F32, tag="acto")
                # the last k-slab matmul and the whole mix chain run per
                # batch half, so half 0's mul/mm/copy/dma pipeline under
                # half 1's
                for hb in range(2):
                    hsl = slice(256 * hb, 256 * (hb + 1))
                    mm(y96[:, hsl], w2h[NK12 - 1], h1m[NK12 - 1][:, hsl],
                       start=False, stop=True)
                    nc.vector.tensor_mul(ymix[:, hsl], y96[:, hsl],
                                         cX[:, hsl])
                    mm(out12[:, hsl], sel12, ymix[:, hsl],
                       start=True, stop=True)
                    nc.scalar.activation(out=acto[:, hsl],
                                         in_=out12[:, hsl], func=AF.Copy)
                    nc.gpsimd.dma_start(out=d_out[:, hsl],
                                        in_=acto[:, hsl])

    nc.finalize()
    return nc


_nc_cache = {}


def _get_nc(bias_const):
    if bias_const not in _nc_cache:
        _nc_cache[bias_const] = _build_nc(bias_const)
    return _nc_cache[bias_const]


def _patch_hook_errors():
    # exceptions inside the neuronx-cc hook are swallowed by the PJRT
    # plugin ("CallFunctionObjArgs: error condition"); print them here
    from concourse import bass2jax

    orig = bass2jax.neuronx_cc_hook
    if getattr(orig, "_err_patched", False):
        return

    def wrapped(*a, **k):
        import traceback

        try:
            return orig(*a, **k)
        except BaseException as e:
            print(getattr(e, "output", ""), file=sys.stderr)
            traceback.print_exc()
            raise

    wrapped._err_patched = True
    bass2jax.neuronx_cc_hook = wrapped


def _pack_z_pairs(w):
    # (E, in, out) -> (128, E/2*out). For each expert pair p, two tiles of
    # (128, out/2): T1 = [top: even expert, first half of m-slices;
    # bottom: odd expert, second half], T2 = the swap — so the top PE row
    # group only ever produces the first half of output banks and the bottom
    # the second half, while both experts cover all output columns.
    z = w[:, :LATENT, :]
    out = z.shape[2]
    h = out // 2
    blk = np.empty((128, E // 2, 2, h), np.float32)
    for p in range(E // 2):
        blk[:LATENT, p, 0] = z[2 * p, :, :h]
        blk[LATENT:, p, 0] = z[2 * p + 1, :, h:]
        blk[:LATENT, p, 1] = z[2 * p + 1, :, :h]
        blk[LATENT:, p, 1] = z[2 * p, :, h:]
    return blk.reshape(128, -1)


def _lip_fold(gw, gc):
    # LipschitzLinear: rows of W scaled so row-wise L1 norm <= softplus(c);
    # depends only on the weights, so fold it on the host.
    lipc = np.logaddexp(0.0, np.float64(gc.reshape(())))
    scale = np.minimum(lipc / np.abs(np.float64(gw)).sum(1), 1.0)
    return (np.float64(gw) * scale[:, None]).astype(np.float32)


def _pack_weights(f, bias_const):
    c = np.ascontiguousarray
    f16 = np.float16

    gate = np.zeros((128, 520), np.float32)
    gw0 = _lip_fold(f["gw0"], f["gc0"]).T  # [320, 128]
    gate[0:64, 0:128] = gw0[0:64]
    gate[:, 128:256] = gw0[64:192]
    gate[:, 256:384] = gw0[192:320]
    gate[:, 384:512] = _lip_fold(f["gw1"], f["gc1"]).T
    gate[:, 512:520] = _lip_fold(f["gw2"], f["gc2"]).T

    selp = np.zeros((E, 1120), np.float32)
    for e in range(E):
        selp[e, 128 * e: 128 * (e + 1)] = 1.0              # sel8
        selp[e, 1024 + np.arange(ACTIONS) * E + e] = 1.0   # sel96, o-major
    gbp = np.zeros((128, 5), np.float32)
    gbp[:, 0] = f["gb0"]
    gbp[:, 1] = f["gb1"]
    gbp[0:E, 2] = f["gb2"]
    if bias_const:
        gbp[:, 3] = f["b0"].flat[0]
        gbp[:, 4] = f["b1"].flat[0]

    # consolidated input template: gate weights + f32 biases as f16 bit
    # pairs + sel12 + (per-core, filled later) xinA/xinB
    inpk = np.zeros((128, 2078), np.float16)
    inpk[:, 0:520] = gate.astype(np.float16)
    inpk[:, 520:530] = np.ascontiguousarray(gbp).view(np.float16)
    inpk[np.arange(NA), 530 + np.arange(NA) // E] = 1.0  # sel12

    w2 = f["w2"]  # (E, 576, 12); l2 outputs packed o-major: col = o*8+e
    w2p = np.zeros((128, 480), np.float32)
    w2p[:, 0:384] = (w2[:, LATENT:, :].reshape(E, NK12, 128, ACTIONS)
                     .transpose(2, 1, 3, 0).reshape(128, -1))
    w2p[0:LATENT, 384:480] = (w2[:, :LATENT, :].transpose(1, 2, 0)
                              .reshape(LATENT, -1))
    w2p[LATENT, 384:480] = f["b2"].T.reshape(-1)  # bias rides the ones row

    out = {
        "inpack_tpl": inpk,
        "selpack": c(selp.astype(f16)),
        "wz0": c((_pack_z_pairs(f["w0"]) * WSCALE).astype(F8E3NP)),
        "wz1": c((_pack_z_pairs(f["w1"]) * WSCALE).astype(F8E3NP)),
        "w2pack": c(w2p.astype(f16)),
        "w0hcat": c((f["w0"][:, LATENT:, :].reshape(E, NK0, 128, HIDDEN)
                     .transpose(2, 0, 1, 3).reshape(128, -1)
                     * WSCALE).astype(F8E3NP)),
        "w1hcat": c((f["w1"][:, LATENT:, :].reshape(E, NK12, 128, HIDDEN)
                     .transpose(2, 0, 1, 3).reshape(128, -1)
                     * WSCALE).astype(F8E3NP)),
        # general-bias path: bias rides a fp16 matmul into the (x64-scaled)
        # PSUM, so it ships pre-scaled
        "bpack": c((np.concatenate([f["b0"], f["b1"]], axis=1)
                    * WSCALE).astype(f16)),
    }
    return out


def kernel(**inputs):
    global LAST_EXEC_NS, LAST_RESULTS
    from concourse import bass_utils

    _patch_hook_errors()

    f = {k: np.ascontiguousarray(np.asarray(v, dtype=np.float32))
         for k, v in inputs.items()}

    bias_const = bool(
        np.all(f["b0"] == f["b0"].flat[0]) and np.all(f["b1"] == f["b1"].flat[0])
    )

    shared = _pack_weights(f, bias_const)
    tpl = shared.pop("inpack_tpl")
    in_maps = []
    for ci in range(NCORES):
        sl = slice(ci * BS, (ci + 1) * BS)
        m = dict(shared)
        inpk = tpl.copy()
        inpk[0:LATENT, 542:1054] = f["z"][sl].T
        inpk[:, 1054:2078] = (f["c"][sl].T.reshape(2, 128, 512)
                              .transpose(1, 0, 2).reshape(128, 1024))
        m["inpack"] = np.ascontiguousarray(inpk)
        in_maps.append(m)

    nc = _get_nc(bias_const)
    res = bass_utils.run_bass_kernel_spmd(
        nc, in_maps, list(range(NCORES)), trace=TRACE
    )
    LAST_EXEC_NS = res.exec_time_ns
    LAST_RESULTS = res
    out = np.concatenate(
        [np.asarray(res.results[ci]["outF"]).T for ci in range(NCORES)],
        axis=0,
    )
    return np.ascontiguousarray(out)


# revision 80
# speedup vs baseline: 1.0216x; 1.0216x over previous
"""Trainium2 Bass kernel for nn_MixedLipMlp (soft-MoE MLP with Lipschitz gate).

Strategy: data-parallel over batch B=4096 across 8 NeuronCores (512 rows each,
expert weights + gate replicated). Activations live feature-major (features on
partitions, batch on the free dim) for layers 0/1; layer 2 stays feature-major
too (out [96, 512] = experts o-major) and the coefficient mix is ONE one-hot
partition-sum matmul, so the output DMA is 12 large packets instead of 128
tiny ones (the v1 batch-major mix cost ~4.3us of DMA tail).

v3 changes vs the 92.7us v1 baseline:
  - layer 2 feature-major + one-hot mix matmul (kills the coeffB machinery,
    the per-bt prod/reduce, and the slow [128, 48] output DMA).
  - constant-bias fast path: the reference initializes expert biases to a
    constant (b = 0.01); when all-equal, coeff @ b == b (softmax sums to 1),
    so the bias folds into the ELU epilogue as a per-partition ACT bias and
    the 8 bias matmuls + bpack DMA disappear. General-bias fallback keeps the
    v1 bias-matmul path.
  - k-major (slab-major) h-passes in layers 0/1: the DVE produces the
    coeff-scaled rhs tiles just-in-time, one k-slab ahead of the PE, instead
    of needing all slabs of an expert up front.
  - mid-layer ELU = exp on ACT + relu on ACT + min on DVE; the final k-group
    of each layer runs m-outer so PSUM banks close staggered and the ELU
    pipeline overlaps the remaining matmuls.
  - DMA: critical inputs (gate weights, z, c) first on sync/scalar; the big
    weight streams are held behind a tiny cross-queue dependency on z so they
    don't contend with the gate-critical transfers, then stream need-ordered
    (w0 before w1 halves, wz1/w2 between).
"""

import os
import sys

if "/opt/trn_rl_repo" not in sys.path:
    sys.path.insert(0, "/opt/trn_rl_repo")

# recover cleanly if a previous process left the NeuronCores wedged
os.environ.setdefault("NEURON_RT_RESET_CORES", "1")

import numpy as np
import ml_dtypes

F8E3NP = ml_dtypes.float8_e3m4
WSCALE = 64.0  # e3m4 weight scaling (undone by the ELU epilogue)

# Problem dimensions (hardcoded; must match the grader's setup_inputs()).
B = 4096
NCORES = 8
BS = B // NCORES  # 512 batch rows per core = matmul free dim
LATENT = 64
INPUT_SIZE = 256
IN_DIM = LATENT + INPUT_SIZE  # 320
HIDDEN = 512
ACTIONS = 12
E = 8
GATE_H = 128
INTER = HIDDEN + LATENT  # 576

NK0 = 2   # layer0: c has 256 rows = 2 k-slabs
NK12 = 4  # layers1,2: h has 512 rows = 4 k-slabs
N_M = HIDDEN // 128  # 4 output m-tiles for layers 0/1
NA = E * ACTIONS  # 96: layer-2 outputs packed o-major (col = o*8+e)

TRACE = False
LAST_EXEC_NS = None
LAST_RESULTS = None


def _build_nc(bias_const):
    import concourse.mybir as mybir
    from concourse import bacc
    from concourse.tile import TileContext

    dt = mybir.dt
    F32 = dt.float32
    F16 = dt.float16
    F8E3 = dt.float8e3
    AF = mybir.ActivationFunctionType
    OP = mybir.AluOpType
    # l0/l1 weights ship as e3m4 (4-bit mantissa) scaled by 64: halves the
    # weight DMA at unchanged PE speed (matmul speed keys on the moving
    # operand's dtype; fp8 lhsT x fp16 rhs is legal), costs ~1e-2 of the
    # 2e-2 error budget. The ELU epilogue unscales with ACT scale=1/64.
    WSCALE_INV = 1.0 / 64.0

    nc = bacc.Bacc("TRN2", target_bir_lowering=False)

    # ---- DRAM I/O ------------------------------------------------------
    # ALL critical inputs ride in ONE consolidated tensor on the gpsimd
    # (SWDGE) queue: sync/scalar DMAs share a single hardware DMA engine
    # (~26GB/s serialized), while SWDGE fans packets across 8 engines.
    # cols 0:542 gatepack: gw0a(0:128,rows<64) gw0b(128:256) gw0c(256:384)
    #   gw1(384:512) gw2(512:520), Lipschitz-folded on the host; 520:530
    #   the five f32 biases (gb0 gb1 gb2 b0c b1c) as f16 bit pairs
    #   (bitcast on device); 530:542 rows<96: sel12.
    # cols 542:1054 rows<64: xinB (zT); 1054:1566 c slab0; 1566:2078 c
    # slab1 (ordered so the first DMA chunk carries everything the gate
    # chain's first matmuls need).
    d_inp = nc.dram_tensor("inpack", [128, 2078], F16, kind="ExternalInput")
    # selpack cols: sel8(0:1024) sel96(1024:1120)
    d_selp = nc.dram_tensor("selpack", [E, 1120], F16, kind="ExternalInput")
    d_wz0 = nc.dram_tensor("wz0", [128, E // 2 * HIDDEN], F8E3,
                           kind="ExternalInput")
    d_wz1 = nc.dram_tensor("wz1", [128, E // 2 * HIDDEN], F8E3,
                           kind="ExternalInput")
    # w2pack cols: w2h slabs (0:384) all 128 rows; w2z+b2 (384:480) rows<65
    d_w2 = nc.dram_tensor("w2pack", [128, 480], F16, kind="ExternalInput")
    d_w0h = nc.dram_tensor("w0hcat", [128, E * NK0 * HIDDEN], F8E3,
                           kind="ExternalInput")
    d_w1h = nc.dram_tensor("w1hcat", [128, E * NK12 * HIDDEN], F8E3,
                           kind="ExternalInput")
    d_bp = nc.dram_tensor("bpack", [E, 2 * HIDDEN], F16, kind="ExternalInput")
    d_out = nc.dram_tensor("outF", [ACTIONS, BS], F32, kind="ExternalOutput")

    mm = nc.tensor.matmul
    SL0 = NK0 * HIDDEN    # 1024 cols per l0 expert slab block
    SL1 = NK12 * HIDDEN   # 2048 cols per l1 expert slab block

    with TileContext(nc) as tc:
        from contextlib import ExitStack

        with ExitStack() as ctx:
            pers = ctx.enter_context(tc.tile_pool(name="pers", bufs=1))
            sca = ctx.enter_context(tc.tile_pool(name="sca", bufs=10))
            etmp = ctx.enter_context(tc.tile_pool(name="etmp", bufs=4))

            # ---- DMA: everything on the gpsimd/SWDGE ring, need-ordered --
            inp = pers.tile([128, 2078], F16, tag="inp")
            nc.gpsimd.dma_start(out=inp[:, 0:1566], in_=d_inp[:, 0:1566])
            nc.gpsimd.dma_start(out=inp[:, 1566:2078],
                                in_=d_inp[:, 1566:2078])
            gate = inp[:, 0:542]
            xinB = inp[0:LATENT, 542:1054]
            xinA = inp[:, 1054:2078]

            # selector one-hots: small 8-packet transfer on the otherwise
            # idle scalar queue
            selp = pers.tile([E, 1120], F16, tag="selp")
            nc.scalar.dma_start(out=selp, in_=d_selp[:, :])
            sel8 = selp[:, 0:1024]
            sel96 = selp[:, 1024:1120]

            w0hcat = pers.tile([128, E * SL0], F8E3, tag="w0hcat")
            w1hcat = pers.tile([128, E * SL1], F8E3, tag="w1hcat")
            wz0 = pers.tile([128, E // 2 * HIDDEN], F8E3, tag="wz0")
            wz1 = pers.tile([128, E // 2 * HIDDEN], F8E3, tag="wz1")
            w2p = pers.tile([128, 480], F16, tag="w2p")

            # weight stream, held until the inputs land (the DMA hw round-
            # robins packets across outstanding transfers, so an early
            # weight stream starves the critical inputs). Tile hoists any
            # dma_start with no dependency, so EVERY gated dma gets its own
            # compute-op touch (RAW on the inputs, WAW into its own output
            # region). Ring order = need order: w0 chunks, z packs, w1
            # expert chunks, w2.
            in8 = inp[0:128, 0:2].bitcast(F8E3)

            def _gp_gated(out_ap, in_ap, touch_sl):
                nc.gpsimd.tensor_copy(out=touch_sl, in_=in8)
                nc.gpsimd.dma_start(out=out_ap, in_=in_ap)

            for c0 in range(4):
                sl = slice(2 * c0 * SL0, 2 * (c0 + 1) * SL0)
                _gp_gated(w0hcat[:, sl], d_w0h[:, sl],
                          w0hcat[0:128, sl.start:sl.start + 4])
            _gp_gated(wz0, d_wz0[:, :], wz0[0:128, 0:4])
            _gp_gated(wz1, d_wz1[:, :], wz1[0:128, 0:4])
            for c1 in range(4):
                sl = slice(2 * c1 * SL1, 2 * (c1 + 1) * SL1)
                _gp_gated(w1hcat[:, sl], d_w1h[:, sl],
                          w1hcat[0:128, sl.start:sl.start + 4])
            nc.gpsimd.tensor_copy(out=w2p[0:128, 0:2],
                                  in_=inp[0:128, 0:2])
            nc.gpsimd.dma_start(out=w2p, in_=d_w2[:, :])
            if not bias_const:
                bp = pers.tile([E, 2 * HIDDEN], F16, tag="bp")
                nc.gpsimd.tensor_copy(out=bp[0:E, 0:2], in_=inp[0:E, 0:2])
                nc.gpsimd.dma_start(out=bp, in_=d_bp[:, :])
                b0sb = bp[:, 0:HIDDEN]
                b1sb = bp[:, HIDDEN:]

            gw0t = [gate[0:64, 0:128], gate[:, 128:256], gate[:, 256:384]]
            gw1t = gate[:, 384:512]
            gw2t = gate[:, 512:520]
            gbp = gate[:, 520:530].bitcast(F32)  # [128, 5] f32 biases
            sel12 = gate[0:NA, 530:542]
            sel8 = selp[:, 0:1024]
            sel96 = selp[:, 1024:1120]
            gb0 = gbp[:, 0:1]
            gb1 = gbp[:, 1:2]
            gb2 = gbp[0:E, 2:3]
            b0c = gbp[:, 3:4]
            b1c = gbp[:, 4:5]
            # 64*b1 as a per-partition f32 scalar, for the DVE-relu variant
            # of the last l1 ELU (relu(64y') = 64*relu(y'), so the x64 PSUM
            # scale folds through the max)
            b1c64 = pers.tile([128, 1], F32, tag="b1c64")
            nc.scalar.activation(out=b1c64, in_=b1c, func=AF.Copy,
                                 scale=WSCALE)
            xc = [xinA[:, 0:512], xinA[:, 512:1024]]
            w0h = [w0hcat[:, e * SL0:(e + 1) * SL0] for e in range(E)]
            w1h = [w1hcat[:, e * SL1:(e + 1) * SL1] for e in range(E)]
            w2h = [w2p[:, k * NA:(k + 1) * NA] for k in range(NK12)]
            w2z = w2p[0:LATENT + 1, 384:480]

            # ---- constants + on-device z expansion -----------------------
            ones_blk = pers.tile([128, 128], F16, tag="ones_blk")
            nc.vector.memset(ones_blk, 1.0)
            warm_rhs = pers.tile([128, BS], F16, tag="warm_rhs")
            nc.vector.memset(warm_rhs, 0.0)
            # z duplicated into both row halves for the zsf scalings; ones
            # row appended for the l2 bias
            xz2 = pers.tile([128, BS], F16, tag="xz2")
            nc.vector.tensor_copy(out=xz2[0:LATENT, :], in_=xinB)
            nc.vector.tensor_copy(out=xz2[LATENT:128, :], in_=xinB)
            xzo = pers.tile([LATENT + 1, BS], F16, tag="xzo")
            nc.vector.tensor_copy(out=xzo[0:LATENT, :], in_=xinB)
            nc.vector.memset(xzo[LATENT:LATENT + 1, :], 1.0)

            # ---- gate + softmax + coefficient broadcasts -----------------
            # the whole chain is column-split into four 128-wide quarters:
            # its serial latency (mm -> elu -> mm -> ... -> coeff) is the
            # longest PE-idle stretch of the kernel, and quarter-stages
            # pipeline across the PE/ACT/DVE engines
            QW = BS // 4
            quarters = [slice(QW * i, QW * (i + 1)) for i in range(4)]
            with tc.tile_pool(name="ps_g", bufs=4, space="PSUM") as ps_g:

                # trip the PE activity monitor before the gate chain; 7
                # fillers keep the PE warm until the inputs land (bufs=2 so
                # they pipeline instead of serializing on the bank drain)
                for _ in range(7):
                    pw = ps_g.tile([128, BS], F32, tag="warm", bufs=2,
                                   name=f"warm{nc.next_id()}")
                    mm(pw, ones_blk, warm_rhs, start=True, stop=True)

                def gate_elu_q(ps, bias, out, sl):
                    # elu(y) = min(exp(y)-1, relu(y)); exp on ACT and relu
                    # on DVE run concurrently (latency-critical chain)
                    ex = etmp.tile([ps.shape[0], QW], F16, tag="elu_exp",
                                   name=f"gex{nc.next_id()}")
                    nc.scalar.activation(out=ex, in_=ps, func=AF.Exp,
                                         bias=bias)
                    rl = etmp.tile([ps.shape[0], QW], F16, tag="elu_relu",
                                   name=f"grl{nc.next_id()}")
                    nc.vector.tensor_scalar(rl, ps, bias, 0.0, OP.add, OP.max)
                    nc.vector.scalar_tensor_tensor(
                        out=out[:, sl], in0=ex, scalar=1.0, in1=rl,
                        op0=OP.subtract, op1=OP.min,
                    )

                h0g = pers.tile([GATE_H, BS], F16, tag="h0g")
                h1g = pers.tile([GATE_H, BS], F16, tag="h1g")
                expl = pers.tile([E, BS], F16, tag="expl")
                bcR = pers.tile([128, BS], F32, tag="bcR")
                coeffT = pers.tile([E, BS], F16, tag="coeffT")
                rhs0 = [xinB, xc[0], xc[1]]
                psg0, psg1, pslg, pssum = [], [], [], []
                for qi, sl in enumerate(quarters):
                    p = ps_g.tile([GATE_H, QW], F32, tag="g", bufs=3,
                                  name=f"psg0{qi}")
                    for k in range(3):
                        mm(p, gw0t[k], rhs0[k][:, sl],
                           start=(k == 0), stop=(k == 2))
                    psg0.append(p)
                for qi, sl in enumerate(quarters):
                    gate_elu_q(psg0[qi], gb0, h0g, sl)
                for qi, sl in enumerate(quarters):
                    p = ps_g.tile([GATE_H, QW], F32, tag="g", bufs=3,
                                  name=f"psg1{qi}")
                    mm(p, gw1t, h0g[:, sl], start=True, stop=True)
                    psg1.append(p)
                for qi, sl in enumerate(quarters):
                    gate_elu_q(psg1[qi], gb1, h1g, sl)
                for qi, sl in enumerate(quarters):
                    p = ps_g.tile([E, QW], F32, tag="lg", name=f"pslg{qi}",
                                  bufs=2)
                    mm(p, gw2t, h1g[:, sl], start=True, stop=True)
                    pslg.append(p)
                # softmax over the 8 expert partitions (logits bounded by the
                # lip constraint, no max subtraction needed)
                for qi, sl in enumerate(quarters):
                    nc.scalar.activation(out=expl[:, sl], in_=pslg[qi],
                                         func=AF.Exp, bias=gb2)
                    p = ps_g.tile([128, QW], F32, tag="sum", name=f"pss{qi}",
                                  bufs=1)
                    mm(p, ones_blk[:E, :], expl[:, sl], start=True, stop=True)
                    pssum.append(p)
                for qi, sl in enumerate(quarters):
                    nc.vector.reciprocal_approx_fast(out=bcR[:, sl],
                                                     in_=pssum[qi])
                    nc.vector.tensor_mul(coeffT[:, sl], expl[:, sl],
                                         bcR[:E, sl])

            # broadcast each normalized coeff row to all 128 partitions.
            # Only the first three are emitted up front; the rest
            # interleave with l0's first expert groups so the PE never
            # waits on the PSUM-pool drain (the ACT copies).
            acc0_ctx = tc.tile_pool(name="ps_acc0", bufs=4, space="PSUM")
            ps_acc0 = acc0_ctx.__enter__()
            bc_ctx = tc.tile_pool(name="ps_bc", bufs=3, space="PSUM")
            ps_bc = bc_ctx.__enter__()
            bcE = []

            def emit_bcE(e):
                pb = ps_bc.tile([128, BS], F32, tag="bc", name=f"pbc{e}",
                                bufs=3)
                mm(pb, sel8[:, 128 * e: 128 * (e + 1)], coeffT,
                   start=True, stop=True)
                t = pers.tile([128, BS], F16, tag=f"bcE{e}")
                nc.scalar.activation(out=t, in_=pb, func=AF.Copy)
                bcE.append(t)

            for e in range(3):
                emit_bcE(e)
            cs = [[None] * E for _ in range(NK0)]
            zsf = [None] * E

            def z_pass(wz, psl, start):
                # row-paired z matmuls: two experts concurrently in disjoint
                # PE row groups; T1/T2 packing swaps experts between groups
                # so each group covers all 4 m-slices (top -> banks {0,1},
                # bottom -> banks {2,3}).
                for p in range(E // 2):
                    for t_ in range(2):
                        base = p * HIDDEN + t_ * 256
                        etop = 2 * p + t_
                        ebot = 2 * p + 1 - t_
                        st = start and p == 0 and t_ == 0
                        for mi in range(2):
                            mm(psl[mi],
                               wz[:LATENT, base + 128 * mi: base + 128 * (mi + 1)],
                               zsf[etop][:LATENT, :],
                               start=st, stop=False)
                            mm(psl[2 + mi],
                               wz[LATENT:, base + 128 * mi: base + 128 * (mi + 1)],
                               zsf[ebot][LATENT:, :],
                               start=st, stop=False)

            def moe_elu(psl_m, bias, out_tag, min_eng=None):
                # elu(y/64 + b) = min(exp(.)-1, relu(.)); exp and relu both
                # on ACT (throughput path; the DVE is loaded with the
                # scalings), min on Pool by default (DVE for the l2-latency-
                # critical last tile). scale=1/64 undoes the e3m4 weight
                # scaling.
                ex = etmp.tile([128, BS], F16, tag="elu_exp",
                               name=f"mex{nc.next_id()}")
                nc.scalar.activation(out=ex, in_=psl_m, func=AF.Exp,
                                     bias=bias if bias is not None else 0.0,
                                     scale=WSCALE_INV)
                rl = etmp.tile([128, BS], F16, tag="elu_relu",
                               name=f"mrl{nc.next_id()}")
                nc.scalar.activation(out=rl, in_=psl_m, func=AF.Relu,
                                     bias=bias if bias is not None else 0.0,
                                     scale=WSCALE_INV)
                h = pers.tile([128, BS], F16, tag=out_tag, name=out_tag)
                (min_eng or nc.vector).scalar_tensor_tensor(
                    out=h, in0=ex, scalar=1.0, in1=rl,
                    op0=OP.subtract, op1=OP.min,
                )
                return h

            def moe_elu_fast(psl_m, bias64, bias, out_tag):
                # latency-optimized variant for the tile the l2 chain waits
                # on: relu on DVE (parallel with the ACT exp) in the scaled
                # domain — relu(64y'+64b) = 64*relu(y'+b)
                ex = etmp.tile([128, BS], F16, tag="elu_exp",
                               name=f"fex{nc.next_id()}")
                nc.scalar.activation(out=ex, in_=psl_m, func=AF.Exp,
                                     bias=bias if bias is not None else 0.0,
                                     scale=WSCALE_INV)
                rl64 = etmp.tile([128, BS], F16, tag="elu_relu",
                                 name=f"fr64{nc.next_id()}")
                if bias64 is not None:
                    nc.vector.tensor_scalar(rl64, psl_m, bias64, 0.0,
                                            OP.add, OP.max)
                else:
                    nc.vector.tensor_scalar(rl64, psl_m, 0.0, None, OP.max)
                rl = etmp.tile([128, BS], F16, tag="elu_relu",
                               name=f"frl{nc.next_id()}")
                nc.vector.tensor_scalar(rl, rl64, WSCALE_INV, None, OP.mult)
                h = pers.tile([128, BS], F16, tag=out_tag, name=out_tag)
                nc.vector.scalar_tensor_tensor(
                    out=h, in0=ex, scalar=1.0, in1=rl,
                    op0=OP.subtract, op1=OP.min,
                )
                return h

            def h_block(wh, hs_tiles, kis, psl, start):
                # e-major slab groups: each scaled rhs tile is consumed
                # right after the DVE makes it
                for idx, ki in enumerate(kis):
                    for e in range(E):
                        for m in range(N_M):
                            mm(psl[m], wh[e][:, ki * HIDDEN + 128 * m:
                                             ki * HIDDEN + 128 * (m + 1)],
                               hs_tiles[ki][e],
                               start=(start and idx == 0 and e == 0),
                               stop=False)

            def h_close(wh, hs_tiles, ki, psl, bsb, bias, htag,
                        last_min_eng=None):
                # final k-group m-outer: banks close staggered so the ELUs
                # pipeline while the remaining banks still accumulate
                hts = []
                for m in range(N_M):
                    for e in range(E):
                        last = e == E - 1
                        if last and bsb is not None:
                            mm(psl[m], wh[e][:, ki * HIDDEN + 128 * m:
                                             ki * HIDDEN + 128 * (m + 1)],
                               hs_tiles[ki][e], start=False, stop=False)
                            mm(psl[m], bsb[:, 128 * m: 128 * (m + 1)], coeffT,
                               start=False, stop=True)
                        else:
                            mm(psl[m], wh[e][:, ki * HIDDEN + 128 * m:
                                             ki * HIDDEN + 128 * (m + 1)],
                               hs_tiles[ki][e], start=False, stop=last)
                    hts.append(moe_elu(psl[m], bias, f"{htag}{m}"))
                return hts

            def h_close_split(wh, hs_tiles, ki, psl, bsb, bias, htag):
                # l1 variant: the LAST bank's close + ELU run per batch
                # half, so the l2 chain (y96-k3 -> mix -> dma), itself
                # half-split, starts ~1.3us earlier on half 0
                hts = []
                for m in range(N_M - 1):
                    for e in range(E):
                        last = e == E - 1
                        if last and bsb is not None:
                            mm(psl[m], wh[e][:, ki * HIDDEN + 128 * m:
                                             ki * HIDDEN + 128 * (m + 1)],
                               hs_tiles[ki][e], start=False, stop=False)
                            mm(psl[m], bsb[:, 128 * m: 128 * (m + 1)], coeffT,
                               start=False, stop=True)
                        else:
                            mm(psl[m], wh[e][:, ki * HIDDEN + 128 * m:
                                             ki * HIDDEN + 128 * (m + 1)],
                               hs_tiles[ki][e], start=False, stop=last)
                    hts.append(moe_elu(psl[m], bias, f"{htag}{m}"))
                m = N_M - 1
                wsl = slice(ki * HIDDEN + 128 * m, ki * HIDDEN + 128 * (m + 1))
                h = pers.tile([128, BS], F16, tag=f"{htag}{m}",
                              name=f"{htag}{m}")
                for hb in range(2):
                    hsl = slice(256 * hb, 256 * (hb + 1))
                    for e in range(E):
                        last = e == E - 1 and bsb is None
                        mm(psl[m][:, hsl], wh[e][:, wsl],
                           hs_tiles[ki][e][:, hsl], start=False, stop=last)
                    if bsb is not None:
                        mm(psl[m][:, hsl], bsb[:, 128 * m: 128 * (m + 1)],
                           coeffT[:, hsl], start=False, stop=True)
                    ex = etmp.tile([128, 256], F16, tag="elu_exp",
                                   name=f"sx{nc.next_id()}")
                    nc.scalar.activation(out=ex, in_=psl[m][:, hsl],
                                         func=AF.Exp,
                                         bias=bias if bias is not None
                                         else 0.0, scale=WSCALE_INV)
                    rl = etmp.tile([128, 256], F16, tag="elu_relu",
                                   name=f"sr{nc.next_id()}")
                    nc.scalar.activation(out=rl, in_=psl[m][:, hsl],
                                         func=AF.Relu,
                                         bias=bias if bias is not None
                                         else 0.0, scale=WSCALE_INV)
                    nc.vector.scalar_tensor_tensor(
                        out=h[:, hsl], in0=ex, scalar=1.0, in1=rl,
                        op0=OP.subtract, op1=OP.min,
                    )
                hts.append(h)
                return hts

            def h_close_e(wh, hs_tiles, ki, psl, bsb, bias, htag):
                # e-major final k-group: each scaled tile is consumed as
                # soon as the DVE makes it (an m-outer close here would
                # demand every expert's tile at once and stall the PE at
                # the DVE's production rate); ELUs bunch at the end.
                for e in range(E):
                    last = e == E - 1 and bsb is None
                    for m in range(N_M):
                        mm(psl[m], wh[e][:, ki * HIDDEN + 128 * m:
                                         ki * HIDDEN + 128 * (m + 1)],
                           hs_tiles[ki][e], start=False, stop=last)
                if bsb is not None:
                    for m in range(N_M):
                        mm(psl[m], bsb[:, 128 * m: 128 * (m + 1)], coeffT,
                           start=False, stop=True)
                return [moe_elu(psl[m], bias, f"{htag}{m}")
                        for m in range(N_M)]

            # ---- MoE layer 0 -------------------------------------------
            # l0 runs c-slab0 FIRST: its expert groups are interleaved (in
            # PE program order) with the remaining bcE broadcasts, the z
            # pass in the middle, c-slab1 last. Scaled-input production
            # order mirrors the consumption order exactly.
            ps_l0 = [ps_acc0.tile([128, BS], F32, tag="acc", name=f"psl0_{m}")
                     for m in range(N_M)]
            for e in range(E):
                if e + 3 < E:
                    emit_bcE(e + 3)
                t = sca.tile([128, BS], F16, tag="s", name=f"c0_{e}")
                nc.vector.tensor_mul(t, xc[0], bcE[e])
                cs[0][e] = t
                for m in range(N_M):
                    mm(ps_l0[m], w0h[e][:, 128 * m: 128 * (m + 1)],
                       cs[0][e], start=(e == 0), stop=False)
            # batch-major coeff for the l2 mix: cX[o*8+e, b] = coeff[e, b]
            pcx = ps_bc.tile([128, BS], F32, tag="bc", name="pcx", bufs=3)
            mm(pcx[0:NA, :], sel96, coeffT, start=True, stop=True)
            cX = pers.tile([NA, BS], F16, tag="cX")
            nc.scalar.activation(out=cX, in_=pcx[0:NA, :], func=AF.Copy)
            bc_ctx.__exit__(None, None, None)

            # coeff-scaled z per expert (z duplicated in both 64-row halves
            # so an expert can ride either PE row group); shared by l0+l1
            for e in range(E):
                t = pers.tile([128, BS], F16, tag=f"zsf{e}")
                nc.vector.tensor_mul(t, xz2, bcE[e])
                zsf[e] = t
            z_pass(wz0, ps_l0, start=False)
            for e in range(E):
                t = sca.tile([128, BS], F16, tag="s", name=f"c1_{e}")
                nc.vector.tensor_mul(t, xc[1], bcE[e])
                cs[1][e] = t
            if bias_const:
                h0m = h_close_e(w0h, cs, 1, ps_l0, None, b0c, "h0m")
            else:
                h0m = h_close_e(w0h, cs, 1, ps_l0, b0sb, None, "h0m")

            # l1 scaled h inputs, k-major
            hs1 = [[None] * E for _ in range(NK12)]
            for ki in range(NK12):
                for e in range(E):
                    t = sca.tile([128, BS], F16, tag="s", name=f"h{ki}_{e}")
                    nc.vector.tensor_mul(t, h0m[ki], bcE[e])
                    hs1[ki][e] = t

            acc1_ctx = tc.tile_pool(name="ps_acc1", bufs=4, space="PSUM")
            ps_acc1 = acc1_ctx.__enter__()
            ps_l1 = [ps_acc1.tile([128, BS], F32, tag="acc", name=f"psl1_{m}")
                     for m in range(N_M)]
            z_pass(wz1, ps_l1, start=True)
            h_block(w1h, hs1, [0, 1, 2], ps_l1, start=False)
            if bias_const:
                h1m = h_close_split(w1h, hs1, 3, ps_l1, None, b1c, "h1m")
            else:
                h1m = h_close_split(w1h, hs1, 3, ps_l1, b1sb, None, "h1m")
            acc1_ctx.__exit__(None, None, None)
            acc0_ctx.__exit__(None, None, None)

            # ---- MoE layer 2, feature-major: y96[o*8+e, b], then the mix
            # is one elementwise multiply + one one-hot partition-sum
            # matmul per batch half (halves pipeline the serial
            # mul->mm->copy->dma tail)
            with tc.tile_pool(name="ps_l2", bufs=1, space="PSUM") as ps_l2:
                y96 = ps_l2.tile([NA, BS], F32, tag="y96", name="y96")
                mm(y96, w2z, xzo, start=True, stop=False)
                for k in range(NK12 - 1):
                    mm(y96, w2h[k], h1m[k], start=False, stop=False)
                ymix = pers.tile([NA, BS], F16, tag="ymix")
                out12 = ps_l2.tile([ACTIONS, BS], F32, tag="o12", name="o12")
                acto = pers.tile([ACTIONS, BS], F32, tag="acto")
                # the last k-slab matmul and the whole mix chain run per
                # batch half, so half 0's mul/mm/copy/dma pipeline under
                # half 1's
                for hb in range(2):
                    hsl = slice(256 * hb, 256 * (hb + 1))
                    mm(y96[:, hsl], w2h[NK12 - 1], h1m[NK12 - 1][:, hsl],
                       start=False, stop=True)
                    nc.vector.tensor_mul(ymix[:, hsl], y96[:, hsl],
                                         cX[:, hsl])
                    mm(out12[:, hsl], sel12, ymix[:, hsl],
                       start=True, stop=True)
                    nc.scalar.activation(out=acto[:, hsl],
                                         in_=out12[:, hsl], func=AF.Copy)
                    nc.gpsimd.dma_start(out=d_out[:, hsl],
                                        in_=acto[:, hsl])

    nc.finalize()
    return nc


_nc_cache = {}


def _get_nc(bias_const):
    if bias_const not in _nc_cache:
        _nc_cache[bias_const] = _build_nc(bias_const)
    return _nc_cache[bias_const]


def _patch_hook_errors():
    # exceptions inside the neuronx-cc hook are swallowed by the PJRT
    # plugin ("CallFunctionObjArgs: error condition"); print them here
    from concourse import bass2jax

    orig = bass2jax.neuronx_cc_hook
    if getattr(orig, "_err_patched", False):
        return

    def wrapped(*a, **k):
        import traceback

        try:
            return orig(*a, **k)
        except BaseException as e:
            print(getattr(e, "output", ""), file=sys.stderr)
            traceback.print_exc()
            raise

    wrapped._err_patched = True
    bass2jax.neuronx_cc_hook = wrapped


def _pack_z_pairs(w):
    # (E, in, out) -> (128, E/2*out). For each expert pair p, two tiles of
    # (128, out/2): T1 = [top: even expert, first half of m-slices;
    # bottom: odd expert, second half], T2 = the swap — so the top PE row
    # group only ever produces the first half of output banks and the bottom
    # the second half, while both experts cover all output columns.
    z = w[:, :LATENT, :]
    out = z.shape[2]
    h = out // 2
    blk = np.empty((128, E // 2, 2, h), np.float32)
    for p in range(E // 2):
        blk[:LATENT, p, 0] = z[2 * p, :, :h]
        blk[LATENT:, p, 0] = z[2 * p + 1, :, h:]
        blk[:LATENT, p, 1] = z[2 * p + 1, :, :h]
        blk[LATENT:, p, 1] = z[2 * p, :, h:]
    return blk.reshape(128, -1)


def _lip_fold(gw, gc):
    # LipschitzLinear: rows of W scaled so row-wise L1 norm <= softplus(c);
    # depends only on the weights, so fold it on the host.
    lipc = np.logaddexp(0.0, np.float64(gc.reshape(())))
    scale = np.minimum(lipc / np.abs(np.float64(gw)).sum(1), 1.0)
    return (np.float64(gw) * scale[:, None]).astype(np.float32)


def _pack_weights(f, bias_const):
    c = np.ascontiguousarray
    f16 = np.float16

    gate = np.zeros((128, 520), np.float32)
    gw0 = _lip_fold(f["gw0"], f["gc0"]).T  # [320, 128]
    gate[0:64, 0:128] = gw0[0:64]
    gate[:, 128:256] = gw0[64:192]
    gate[:, 256:384] = gw0[192:320]
    gate[:, 384:512] = _lip_fold(f["gw1"], f["gc1"]).T
    gate[:, 512:520] = _lip_fold(f["gw2"], f["gc2"]).T

    selp = np.zeros((E, 1120), np.float32)
    for e in range(E):
        selp[e, 128 * e: 128 * (e + 1)] = 1.0              # sel8
        selp[e, 1024 + np.arange(ACTIONS) * E + e] = 1.0   # sel96, o-major
    gbp = np.zeros((128, 5), np.float32)
    gbp[:, 0] = f["gb0"]
    gbp[:, 1] = f["gb1"]
    gbp[0:E, 2] = f["gb2"]
    if bias_const:
        gbp[:, 3] = f["b0"].flat[0]
        gbp[:, 4] = f["b1"].flat[0]

    # consolidated input template: gate weights + f32 biases as f16 bit
    # pairs + sel12 + (per-core, filled later) xinA/xinB
    inpk = np.zeros((128, 2078), np.float16)
    inpk[:, 0:520] = gate.astype(np.float16)
    inpk[:, 520:530] = np.ascontiguousarray(gbp).view(np.float16)
    inpk[np.arange(NA), 530 + np.arange(NA) // E] = 1.0  # sel12

    w2 = f["w2"]  # (E, 576, 12); l2 outputs packed o-major: col = o*8+e
    w2p = np.zeros((128, 480), np.float32)
    w2p[:, 0:384] = (w2[:, LATENT:, :].reshape(E, NK12, 128, ACTIONS)
                     .transpose(2, 1, 3, 0).reshape(128, -1))
    w2p[0:LATENT, 384:480] = (w2[:, :LATENT, :].transpose(1, 2, 0)
                              .reshape(LATENT, -1))
    w2p[LATENT, 384:480] = f["b2"].T.reshape(-1)  # bias rides the ones row

    out = {
        "inpack_tpl": inpk,
        "selpack": c(selp.astype(f16)),
        "wz0": c((_pack_z_pairs(f["w0"]) * WSCALE).astype(F8E3NP)),
        "wz1": c((_pack_z_pairs(f["w1"]) * WSCALE).astype(F8E3NP)),
        "w2pack": c(w2p.astype(f16)),
        "w0hcat": c((f["w0"][:, LATENT:, :].reshape(E, NK0, 128, HIDDEN)
                     .transpose(2, 0, 1, 3).reshape(128, -1)
                     * WSCALE).astype(F8E3NP)),
        "w1hcat": c((f["w1"][:, LATENT:, :].reshape(E, NK12, 128, HIDDEN)
                     .transpose(2, 0, 1, 3).reshape(128, -1)
                     * WSCALE).astype(F8E3NP)),
        # general-bias path: bias rides a fp16 matmul into the (x64-scaled)
        # PSUM, so it ships pre-scaled
        "bpack": c((np.concatenate([f["b0"], f["b1"]], axis=1)
                    * WSCALE).astype(f16)),
    }
    return out


def kernel(**inputs):
    global LAST_EXEC_NS, LAST_RESULTS
    from concourse import bass_utils

    _patch_hook_errors()

    f = {k: np.ascontiguousarray(np.asarray(v, dtype=np.float32))
         for k, v in inputs.items()}

    bias_const = bool(
        np.all(f["b0"] == f["b0"].flat[0]) and np.all(f["b1"] == f["b1"].flat[0])
    )

    shared = _pack_weights(f, bias_const)
    tpl = shared.pop("inpack_tpl")
    in_maps = []
    for ci in range(NCORES):
        sl = slice(ci * BS, (ci + 1) * BS)
        m = dict(shared)
        inpk = tpl.copy()
        inpk[0:LATENT, 542:1054] = f["z"][sl].T
        inpk[:, 1054:2078] = (f["c"][sl].T.reshape(2, 128, 512)
                              .transpose(1, 0, 2).reshape(128, 1024))
        m["inpack"] = np.ascontiguousarray(inpk)
        in_maps.append(m)

    nc = _get_nc(bias_const)
    res = bass_utils.run_bass_kernel_spmd(
        nc, in_maps, list(range(NCORES)), trace=TRACE
    )
    LAST_EXEC_NS = res.exec_time_ns
    LAST_RESULTS = res
    out = np.concatenate(
        [np.asarray(res.results[ci]["outF"]).T for ci in range(NCORES)],
        axis=0,
    )
    return np.ascontiguousarray(out)
